# revision 6
# baseline (speedup 1.0000x reference)
"""MultiHeadAttention (B=2, S=2048, D=1024, H=16) on 8 Trainium2 NeuronCores.

Sharding: core c -> batch b = c // 4, head group g = c % 4 (4 of 16 heads =
256 of the 1024 projection columns). Within a batch's 4-core group:

  P1-P2: q/k projections for the core's 4 heads over the full sequence,
         produced directly transposed/head-major: qT,kT [256, S].
  P3:    v projection in natural layout [S, 4*65] fp16, with a ones column
         appended per head (yields softmax denominators for free in P4).
  P4:    per head: scoresT[k,q] = kT_blk.T @ qT (f32r), exp on ScalarE with
         the 1/sqrt(64) scale folded in (scores are O(10) so no max pass),
         then oT'[65,q] += v_blk.T @ pT (fp16 operands, fp32 PSUM accum).
         Row 64 of oT' is the softmax sum; normalize via reciprocal +
         PE broadcast, giving the context ctxT [256, S].
  P5:    partial output projection over the full sequence through the
         core's 256 context rows (+ bo/4 so the group sum adds bo once).
  P6:    ReduceScatter(add) across the 4-core group; rank r receives the
         summed rows [512r, 512r+512) = exactly this core's output shard.

All heavy matmuls run as float32r (full PE rate at N>=256, ~1e-4 rel err).
"""

import numpy as np

import concourse.bacc as bacc
import concourse.mybir as mybir
from concourse.tile import TileContext
from concourse.bass_utils import run_bass_kernel_spmd

F32 = mybir.dt.float32
F32R = mybir.dt.float32r
F16 = mybir.dt.float16

B, S, D = 2, 2048, 1024
H, DH = 16, 64
NCORES = 8
GROUPS = [[0, 1, 2, 3], [4, 5, 6, 7]]
HPG = 4            # heads per core
DG = HPG * DH      # 256 projection cols per core
QCH = S // 4       # 512 output rows per core
IC = D // 128      # 8 contraction chunks for the projections
KC = S // 128      # 16 key blocks
VW = DH + 1        # 65 = head dim + ones column

_NC_CACHE = {}
_ONES = np.ones((1, DH), np.float32)


def _build_nc():
    nc = bacc.Bacc("TRN2", target_bir_lowering=False, num_devices=NCORES)

    xq = nc.dram_tensor("xq", [IC, 128, S], F32R, kind="ExternalInput")
    xk = nc.dram_tensor("xk", [IC, 128, S], F32R, kind="ExternalInput")
    xv = nc.dram_tensor("xv", [IC, 128, S], F32R, kind="ExternalInput")
    wq = nc.dram_tensor("wq", [IC, 128, DG], F32R, kind="ExternalInput")
    wk = nc.dram_tensor("wk", [IC, 128, DG], F32R, kind="ExternalInput")
    wv = nc.dram_tensor("wv", [IC, 128, DG], F32R, kind="ExternalInput")
    wo = nc.dram_tensor("wo", [2, 128, D], F32R, kind="ExternalInput")
    bq2 = nc.dram_tensor("bq2", [2, 128], F32, kind="ExternalInput")
    bk2 = nc.dram_tensor("bk2", [2, 128], F32, kind="ExternalInput")
    bvb = nc.dram_tensor("bvb", [128, DG], F32, kind="ExternalInput")
    bob = nc.dram_tensor("bob", [128, D], F32, kind="ExternalInput")
    ones1 = nc.dram_tensor("ones1", [1, DH], F32R, kind="ExternalInput")
    out = nc.dram_tensor("out", [QCH, D], F32, kind="ExternalOutput")

    partial = nc.dram_tensor("partial", [S, D], F32)
    rs_out = nc.dram_tensor("rs_out", [QCH, D], F32)

    Exp = mybir.ActivationFunctionType.Exp

    with nc.allow_low_precision(reason="f32r/f16 attention internals"), \
            TileContext(nc) as tc:
        with (
            tc.tile_pool(name="persist", bufs=1) as pers,
            tc.tile_pool(name="xin", bufs=3) as xin,
            tc.tile_pool(name="pt", bufs=4) as ptp,
            tc.tile_pool(name="small", bufs=4) as small,
            tc.tile_pool(name="outp", bufs=3) as outp,
            tc.tile_pool(name="ps", bufs=3, space="PSUM") as ps,
            tc.tile_pool(name="psov", bufs=2, space="PSUM") as psov,
            tc.tile_pool(name="psbc", bufs=2, space="PSUM") as psbc,
        ):
            # ---- persistent SBUF ----
            wq_sb = pers.tile([128, IC * DG], F32R, tag="wq")
            wk_sb = pers.tile([128, IC * DG], F32R, tag="wk")
            wv_sb = pers.tile([128, IC * DG], F32R, tag="wv")
            wo_sb = pers.tile([128, 2 * D], F32R, tag="wo")
            qt_sb = [pers.tile([128, S], F32R, tag=f"qt{i}", name=f"qt{i}") for i in range(2)]
            kt_sb = [pers.tile([128, S], F32R, tag=f"kt{i}", name=f"kt{i}") for i in range(2)]
            v_sb = pers.tile([128, KC * HPG * VW], F16, tag="v")
            ctxT_sb = [pers.tile([128, S], F32R, tag=f"cx{i}", name=f"cx{i}") for i in range(2)]
            bq_sb = pers.tile([128, 2], F32, tag="bq")
            bk_sb = pers.tile([128, 2], F32, tag="bk")
            bvb_sb = pers.tile([128, DG], F32, tag="bvb")
            bob_sb = pers.tile([128, D], F32, tag="bob")
            ones_sb = pers.tile([1, DH], F32R, tag="ones")

            for wsb, wdr, kchunks in (
                (wq_sb, wq, IC), (wk_sb, wk, IC), (wv_sb, wv, IC), (wo_sb, wo, 2),
            ):
                nc.sync.dma_start(
                    out=wsb.rearrange("p (k n) -> p k n", k=kchunks),
                    in_=wdr.rearrange("k p n -> p k n"),
                )
            nc.sync.dma_start(out=bq_sb[:], in_=bq2.rearrange("c p -> p c"))
            nc.sync.dma_start(out=bk_sb[:], in_=bk2.rearrange("c p -> p c"))
            nc.sync.dma_start(out=bvb_sb[:], in_=bvb[:])
            nc.sync.dma_start(out=bob_sb[:], in_=bob[:])
            nc.sync.dma_start(out=ones_sb[:], in_=ones1[:])
            nc.vector.memset(
                v_sb.rearrange("p (k h e) -> p k h e", h=HPG, e=VW)[:, :, :, DH], 1.0
            )

            # ---- P1/P2: q and k projections -> head-major [256, S] ----
            for xdr, wsb, bsb, dst in (
                (xq, wq_sb, bq_sb, qt_sb),
                (xk, wk_sb, bk_sb, kt_sb),
            ):
                wsb3 = wsb.rearrange("p (k n) -> p k n", k=IC)
                for s4 in range(4):  # 512-wide sequence slice
                    xt = xin.tile([128, IC * 512], F32R, tag="x")
                    nc.sync.dma_start(
                        out=xt.rearrange("p (k n) -> p k n", k=IC),
                        in_=xdr[:, :, s4 * 512:(s4 + 1) * 512].rearrange(
                            "k p n -> p k n"),
                    )
                    xt3 = xt.rearrange("p (k n) -> p k n", k=IC)
                    for oc in range(2):
                        acc = ps.tile([128, 512], F32, tag="mm")
                        for ic in range(IC):
                            nc.tensor.matmul(
                                acc[:],
                                wsb3[:, ic, oc * 128:(oc + 1) * 128],
                                xt3[:, ic, :],
                                start=(ic == 0),
                                stop=(ic == IC - 1),
                            )
                        nc.vector.tensor_scalar_add(
                            dst[oc][:, s4 * 512:(s4 + 1) * 512],
                            acc[:],
                            bsb[:, oc:oc + 1],
                        )

            # ---- P3: v projection -> [S, 4*65] fp16 with ones columns ----
            wv3 = wv_sb.rearrange("p (k n) -> p k n", k=IC)
            v4 = v_sb.rearrange("p (k h e) -> p k h e", h=HPG, e=VW)
            for s4 in range(4):
                xt = xin.tile([128, IC * 512], F32R, tag="x")
                nc.sync.dma_start(
                    out=xt.rearrange("p (k n) -> p k n", k=IC),
                    in_=xv[:, :, s4 * 512:(s4 + 1) * 512].rearrange("k p n -> p k n"),
                )
                xt3 = xt.rearrange("p (k n) -> p k n", k=IC)
                for j in range(4):  # key chunk kc = 4*s4 + j
                    kc = 4 * s4 + j
                    acc = ps.tile([128, 512], F32, tag="mm")
                    for ic in range(IC):
                        nc.tensor.matmul(
                            acc[:, 0:DG],
                            xt3[:, ic, j * 128:(j + 1) * 128],
                            wv3[:, ic, :],
                            start=(ic == 0),
                            stop=(ic == IC - 1),
                        )
                    nc.vector.tensor_add(
                        out=v4[:, kc, :, 0:DH],
                        in0=acc[:, 0:DG].rearrange("p (h e) -> p h e", e=DH),
                        in1=bvb_sb.rearrange("p (h e) -> p h e", e=DH),
                    )

            # ---- P4: attention per head ----
            for hg in range(HPG):
                oc, ofs = hg // 2, 64 * (hg % 2)
                for qs in range(4):  # 512-wide query slice
                    ov = psov.tile([VW, 512], F32, tag="ov")
                    for kb in range(KC):
                        sc = ps.tile([128, 512], F32, tag="mm")
                        nc.tensor.matmul(
                            sc[:],
                            kt_sb[oc][ofs:ofs + DH, kb * 128:(kb + 1) * 128],
                            qt_sb[oc][ofs:ofs + DH, qs * 512:(qs + 1) * 512],
                            start=True,
                            stop=True,
                        )
                        pt = ptp.tile([128, 512], F16, tag="pt")
                        nc.scalar.activation(pt[:], sc[:], Exp, scale=0.125)
                        nc.tensor.matmul(
                            ov[:],
                            v_sb[:, (kb * HPG + hg) * VW:(kb * HPG + hg + 1) * VW],
                            pt[:],
                            start=(kb == 0),
                            stop=(kb == KC - 1),
                        )
                    recip = small.tile([1, 512], F32R, tag="rc")
                    nc.vector.reciprocal(recip[:], ov[DH:VW, :])
                    bc = psbc.tile([DH, 512], F32, tag="bc")
                    nc.tensor.matmul(bc[:], ones_sb[:], recip[:], start=True, stop=True)
                    bcs = small.tile([DH, 512], F32, tag="bcs")
                    nc.vector.tensor_copy(out=bcs[:], in_=bc[:])
                    nc.vector.tensor_mul(
                        out=ctxT_sb[oc][ofs:ofs + DH, qs * 512:(qs + 1) * 512],
                        in0=ov[0:DH, :],
                        in1=bcs[:],
                    )

            # ---- P5: partial output projection over the full sequence ----
            wo3 = wo_sb.rearrange("p (k n) -> p k n", k=2)
            for ib in range(KC):  # 128-row block of the sequence
                for oh in range(2):  # 512-wide output column half
                    acc = ps.tile([128, 512], F32, tag="mm")
                    for cc in range(2):
                        nc.tensor.matmul(
                            acc[:],
                            ctxT_sb[cc][:, ib * 128:(ib + 1) * 128],
                            wo3[:, cc, oh * 512:(oh + 1) * 512],
                            start=(cc == 0),
                            stop=(cc == 1),
                        )
                    ot = outp.tile([128, 512], F32, tag="ot")
                    nc.vector.tensor_add(
                        out=ot[:], in0=acc[:], in1=bob_sb[:, oh * 512:(oh + 1) * 512]
                    )
                    nc.sync.dma_start(
                        out=partial[ib * 128:(ib + 1) * 128,
                                    oh * 512:(oh + 1) * 512],
                        in_=ot[:],
                    )

            # ---- P6: sum partials across the group; rank r keeps its shard ----
            nc.gpsimd.collective_compute(
                "ReduceScatter",
                mybir.AluOpType.add,
                replica_groups=GROUPS,
                ins=[partial[:]],
                outs=[rs_out[:]],
            )
            nc.sync.dma_start(out=out[:], in_=rs_out[:])

    nc.compile()
    return nc


def _get_nc():
    if "nc" not in _NC_CACHE:
        _NC_CACHE["nc"] = _build_nc()
    return _NC_CACHE["nc"]


def _prep_inputs(Q, K, V, Wq, Wk, Wv, Wo, bq, bk, bv, bo):
    f = np.float32
    Q, K, V = (np.ascontiguousarray(np.asarray(a, f)) for a in (Q, K, V))
    Wq, Wk, Wv, Wo = (np.asarray(a, f) for a in (Wq, Wk, Wv, Wo))
    bq, bk, bv, bo = (np.asarray(a, f) for a in (bq, bk, bv, bo))

    xqs = [np.ascontiguousarray(Q[b].T).reshape(IC, 128, S) for b in range(B)]
    xks = [np.ascontiguousarray(K[b].T).reshape(IC, 128, S) for b in range(B)]
    xvs = [np.ascontiguousarray(V[b].T).reshape(IC, 128, S) for b in range(B)]
    WqT, WkT, WvT, WoT = Wq.T, Wk.T, Wv.T, Wo.T
    bob = np.ascontiguousarray(
        np.broadcast_to(bo / 4.0, (128, D)), dtype=f)

    in_maps = []
    for c in range(NCORES):
        b, g = c // 4, c % 4
        cols = slice(DG * g, DG * (g + 1))
        in_maps.append({
            "xq": xqs[b], "xk": xks[b], "xv": xvs[b],
            "wq": np.ascontiguousarray(WqT[:, cols]).reshape(IC, 128, DG),
            "wk": np.ascontiguousarray(WkT[:, cols]).reshape(IC, 128, DG),
            "wv": np.ascontiguousarray(WvT[:, cols]).reshape(IC, 128, DG),
            "wo": np.ascontiguousarray(WoT[cols, :]).reshape(2, 128, D),
            "bq2": np.ascontiguousarray(bq[cols]).reshape(2, 128),
            "bk2": np.ascontiguousarray(bk[cols]).reshape(2, 128),
            "bvb": np.ascontiguousarray(np.broadcast_to(bv[cols], (128, DG))),
            "bob": bob,
            "ones1": np.ones((1, KK_DH), np.float32) if False else _ONES,
        })
    return in_maps


def kernel(**inputs):
    nc = _get_nc()
    in_maps = _prep_inputs(**inputs)
    res = run_bass_kernel_spmd(nc, in_maps, core_ids=list(range(NCORES)))
    out = np.empty((B, S, D), np.float32)
    for c in range(NCORES):
        b, g = c // 4, c % 4
        out[b, QCH * g:QCH * (g + 1), :] = res.results[c]["out"]
    return out


# revision 8
# speedup vs baseline: 1.4243x; 1.4243x over previous
"""MultiHeadAttention (B=2, S=2048, D=1024, H=16) on 8 Trainium2 NeuronCores.

Sharding: core c -> batch b = c // 4, head group g = c % 4 (4 of 16 heads =
256 of the 1024 projection columns). Within a batch's 4-core group:

  P1-P2: q/k projections for the core's 4 heads over the full sequence,
         produced directly transposed/head-major: qT,kT [256, S] fp16.
  P3:    v projection in natural layout [S, 4*65] fp16, with a ones column
         appended per head (yields softmax denominators for free in P4).
  P4:    per head and 1024-wide query slice: scoresT[k,q] = kT_blk.T @ qT
         (fp16 operands, fp32 PSUM), one 1024-wide exp on ScalarE with the
         1/sqrt(64) scale folded in (scores are O(10): no max pass needed),
         then oT'[65,q] += v_blk.T @ pT. Row 64 of oT' is the softmax sum;
         normalize via reciprocal + PE broadcast -> ctxT [256, S] fp16.
  P5:    partial output projection for the finished 1024 query rows
         (+ bo/4 so the group sum applies bo exactly once).
  P6:    ReduceScatter(add) in 512-row chunks, pipelined behind P4/P5 so
         only the last chunk's exchange is serial; rank r of the group
         receives rows [512j + 128r, +128) of chunk j.

Matmul operands are fp16 (1 cycle/row on the PE; same 10-bit mantissa class
as the fp32r mode, which measures ~2 cycles/row); all accumulation is fp32.
"""

import numpy as np

import concourse.bacc as bacc
import concourse.mybir as mybir
from concourse.tile import TileContext
from concourse.bass_utils import run_bass_kernel_spmd

F32 = mybir.dt.float32
F32R = mybir.dt.float32r
F16 = mybir.dt.float16

B, S, D = 2, 2048, 1024
H, DH = 16, 64
NCORES = 8
GROUPS = [[0, 1, 2, 3], [4, 5, 6, 7]]
HPG = 4            # heads per core
DG = HPG * DH      # 256 projection cols per core
IC = D // 128      # 8 contraction chunks for the projections
KC = S // 128      # 16 key blocks
VW = DH + 1        # 65 = head dim + ones column

_NC_CACHE = {}
_ONES = np.ones((1, DH), np.float32)


def _build_nc():
    nc = bacc.Bacc("TRN2", target_bir_lowering=False, num_devices=NCORES)

    xq = nc.dram_tensor("xq", [IC, 128, S], F16, kind="ExternalInput")
    xk = nc.dram_tensor("xk", [IC, 128, S], F16, kind="ExternalInput")
    xv = nc.dram_tensor("xv", [IC, 128, S], F16, kind="ExternalInput")
    wq = nc.dram_tensor("wq", [IC, 128, DG], F16, kind="ExternalInput")
    wk = nc.dram_tensor("wk", [IC, 128, DG], F16, kind="ExternalInput")
    wv = nc.dram_tensor("wv", [IC, 128, DG], F16, kind="ExternalInput")
    wo = nc.dram_tensor("wo", [2, 128, D], F16, kind="ExternalInput")
    bq2 = nc.dram_tensor("bq2", [2, 128], F32, kind="ExternalInput")
    bk2 = nc.dram_tensor("bk2", [2, 128], F32, kind="ExternalInput")
    bvb = nc.dram_tensor("bvb", [128, DG], F32, kind="ExternalInput")
    bob = nc.dram_tensor("bob", [128, D], F32, kind="ExternalInput")
    ones1 = nc.dram_tensor("ones1", [1, DH], F32R, kind="ExternalInput")
    out = nc.dram_tensor("out", [4, 128, D], F32, kind="ExternalOutput")

    partial = nc.dram_tensor("partial", [S, D], F32)
    rs_out = nc.dram_tensor("rs_out", [4, 128, D], F32)

    Exp = mybir.ActivationFunctionType.Exp

    with nc.allow_low_precision(reason="fp16 attention internals"), \
            TileContext(nc) as tc:
        with (
            tc.tile_pool(name="persist", bufs=1) as pers,
            tc.tile_pool(name="xin", bufs=3) as xin,
            tc.tile_pool(name="pt", bufs=3) as ptp,
            tc.tile_pool(name="small", bufs=4) as small,
            tc.tile_pool(name="outp", bufs=3) as outp,
            tc.tile_pool(name="ps", bufs=2, space="PSUM") as ps,
            tc.tile_pool(name="ps2", bufs=2, space="PSUM") as ps2,
            tc.tile_pool(name="psov", bufs=1, space="PSUM") as psov,
        ):
            # ---- persistent SBUF ----
            wq_sb = pers.tile([128, IC * DG], F16, tag="wq")
            wk_sb = pers.tile([128, IC * DG], F16, tag="wk")
            wv_sb = pers.tile([128, IC * DG], F16, tag="wv")
            wo_sb = pers.tile([128, 2 * D], F16, tag="wo")
            qt_sb = [pers.tile([128, S], F16, tag=f"qt{i}", name=f"qt{i}")
                     for i in range(2)]
            kt_sb = [pers.tile([128, S], F16, tag=f"kt{i}", name=f"kt{i}")
                     for i in range(2)]
            v_sb = pers.tile([128, KC * HPG * VW], F16, tag="v")
            ctxT_sb = [pers.tile([128, S], F16, tag=f"cx{i}", name=f"cx{i}")
                       for i in range(2)]
            bq_sb = pers.tile([128, 2], F32, tag="bq")
            bk_sb = pers.tile([128, 2], F32, tag="bk")
            bvb_sb = pers.tile([128, DG], F32, tag="bvb")
            bob_sb = pers.tile([128, D], F32, tag="bob")
            ones_sb = pers.tile([1, DH], F32R, tag="ones")

            for wsb, wdr, kchunks in (
                (wq_sb, wq, IC), (wk_sb, wk, IC), (wv_sb, wv, IC), (wo_sb, wo, 2),
            ):
                nc.sync.dma_start(
                    out=wsb.rearrange("p (k n) -> p k n", k=kchunks),
                    in_=wdr.rearrange("k p n -> p k n"),
                )
            nc.sync.dma_start(out=bq_sb[:], in_=bq2.rearrange("c p -> p c"))
            nc.sync.dma_start(out=bk_sb[:], in_=bk2.rearrange("c p -> p c"))
            nc.sync.dma_start(out=bvb_sb[:], in_=bvb[:])
            nc.sync.dma_start(out=bob_sb[:], in_=bob[:])
            nc.sync.dma_start(out=ones_sb[:], in_=ones1[:])
            nc.vector.memset(
                v_sb.rearrange("p (k h e) -> p k h e", h=HPG, e=VW)[:, :, :, DH], 1.0
            )

            # ---- P1/P2: k then q projections -> head-major [256, S] fp16 ----
            for xdr, wsb, bsb, dst in (
                (xk, wk_sb, bk_sb, kt_sb),
                (xq, wq_sb, bq_sb, qt_sb),
            ):
                wsb3 = wsb.rearrange("p (k n) -> p k n", k=IC)
                for s4 in range(4):  # 512-wide sequence slice
                    xt = xin.tile([128, IC * 512], F16, tag="x")
                    nc.sync.dma_start(
                        out=xt.rearrange("p (k n) -> p k n", k=IC),
                        in_=xdr[:, :, s4 * 512:(s4 + 1) * 512].rearrange(
                            "k p n -> p k n"),
                    )
                    xt3 = xt.rearrange("p (k n) -> p k n", k=IC)
                    for oc in range(2):
                        acc = ps.tile([128, 512], F32, tag="mm")
                        for ic in range(IC):
                            nc.tensor.matmul(
                                acc[:],
                                wsb3[:, ic, oc * 128:(oc + 1) * 128],
                                xt3[:, ic, :],
                                start=(ic == 0),
                                stop=(ic == IC - 1),
                            )
                        nc.vector.tensor_scalar_add(
                            dst[oc][:, s4 * 512:(s4 + 1) * 512],
                            acc[:],
                            bsb[:, oc:oc + 1],
                        )

            # ---- P3: v projection -> [S, 4*65] fp16 with ones columns ----
            wv3 = wv_sb.rearrange("p (k n) -> p k n", k=IC)
            v4 = v_sb.rearrange("p (k h e) -> p k h e", h=HPG, e=VW)
            for s4 in range(4):
                xt = xin.tile([128, IC * 512], F16, tag="x")
                nc.sync.dma_start(
                    out=xt.rearrange("p (k n) -> p k n", k=IC),
                    in_=xv[:, :, s4 * 512:(s4 + 1) * 512].rearrange("k p n -> p k n"),
                )
                xt3 = xt.rearrange("p (k n) -> p k n", k=IC)
                for j in range(4):  # key chunk kc = 4*s4 + j
                    kc = 4 * s4 + j
                    acc = ps.tile([128, 512], F32, tag="mm")
                    for ic in range(IC):
                        nc.tensor.matmul(
                            acc[:, 0:DG],
                            xt3[:, ic, j * 128:(j + 1) * 128],
                            wv3[:, ic, :],
                            start=(ic == 0),
                            stop=(ic == IC - 1),
                        )
                    nc.vector.tensor_add(
                        out=v4[:, kc, :, 0:DH],
                        in0=acc[:, 0:DG].rearrange("p (h e) -> p h e", e=DH),
                        in1=bvb_sb.rearrange("p (h e) -> p h e", e=DH),
                    )

            # ---- P4 + P5 + P6, pipelined per 1024-wide query slice ----
            wo3 = wo_sb.rearrange("p (k n) -> p k n", k=2)
            for q2 in range(2):  # 1024-wide query slice
                qlo = q2 * 1024
                for hg in range(HPG):
                    oc, ofs = hg // 2, 64 * (hg % 2)
                    ovs = [psov.tile([VW, 512], F32, tag=f"ov{i}", name=f"ov{i}")
                           for i in range(2)]
                    for kb in range(KC):
                        sc = ps2.tile([128, 1024], F32, tag="sc")
                        for half in range(2):
                            nc.tensor.matmul(
                                sc[:, half * 512:(half + 1) * 512],
                                kt_sb[oc][ofs:ofs + DH, kb * 128:(kb + 1) * 128],
                                qt_sb[oc][ofs:ofs + DH,
                                          qlo + half * 512:qlo + (half + 1) * 512],
                                start=True,
                                stop=True,
                            )
                        pt = ptp.tile([128, 1024], F16, tag="pt")
                        nc.scalar.activation(pt[:], sc[:], Exp, scale=0.125)
                        for half in range(2):
                            nc.tensor.matmul(
                                ovs[half][:],
                                v_sb[:, (kb * HPG + hg) * VW:
                                     (kb * HPG + hg + 1) * VW],
                                pt[:, half * 512:(half + 1) * 512],
                                start=(kb == 0),
                                stop=(kb == KC - 1),
                            )
                    for half in range(2):
                        ov = ovs[half]
                        recip = small.tile([1, 512], F32R, tag="rc")
                        nc.vector.reciprocal(recip[:], ov[DH:VW, :])
                        bc = ps.tile([DH, 512], F32, tag="mm")
                        nc.tensor.matmul(bc[:], ones_sb[:], recip[:],
                                         start=True, stop=True)
                        bcs = small.tile([DH, 512], F32, tag="bcs")
                        nc.vector.tensor_copy(out=bcs[:], in_=bc[:])
                        nc.vector.tensor_mul(
                            out=ctxT_sb[oc][ofs:ofs + DH,
                                            qlo + half * 512:qlo + (half + 1) * 512],
                            in0=ov[0:DH, :],
                            in1=bcs[:],
                        )

                # P5: output projection for the finished rows, then
                # ReduceScatter per 512-row chunk.
                for sub in range(2):
                    j = 2 * q2 + sub  # 512-row chunk index
                    for ibl in range(4):
                        ib = 4 * j + ibl
                        for oh in range(2):
                            acc = ps.tile([128, 512], F32, tag="mm")
                            for cc in range(2):
                                nc.tensor.matmul(
                                    acc[:],
                                    ctxT_sb[cc][:, ib * 128:(ib + 1) * 128],
                                    wo3[:, cc, oh * 512:(oh + 1) * 512],
                                    start=(cc == 0),
                                    stop=(cc == 1),
                                )
                            ot = outp.tile([128, 512], F32, tag="ot")
                            nc.vector.tensor_add(
                                out=ot[:], in0=acc[:],
                                in1=bob_sb[:, oh * 512:(oh + 1) * 512],
                            )
                            nc.sync.dma_start(
                                out=partial[ib * 128:(ib + 1) * 128,
                                            oh * 512:(oh + 1) * 512],
                                in_=ot[:],
                            )
                    nc.gpsimd.collective_compute(
                        "ReduceScatter",
                        mybir.AluOpType.add,
                        replica_groups=GROUPS,
                        ins=[partial[j * 512:(j + 1) * 512, :]],
                        outs=[rs_out[j]],
                    )
                    nc.sync.dma_start(out=out[j], in_=rs_out[j])

    nc.compile()
    return nc


def _get_nc():
    if "nc" not in _NC_CACHE:
        _NC_CACHE["nc"] = _build_nc()
    return _NC_CACHE["nc"]


def _prep_inputs(Q, K, V, Wq, Wk, Wv, Wo, bq, bk, bv, bo):
    f = np.float32
    h = np.float16
    Q, K, V = (np.asarray(a, f) for a in (Q, K, V))
    Wq, Wk, Wv, Wo = (np.asarray(a, f) for a in (Wq, Wk, Wv, Wo))
    bq, bk, bv, bo = (np.asarray(a, f) for a in (bq, bk, bv, bo))

    xqs = [np.ascontiguousarray(Q[b].T).astype(h).reshape(IC, 128, S)
           for b in range(B)]
    xks = [np.ascontiguousarray(K[b].T).astype(h).reshape(IC, 128, S)
           for b in range(B)]
    xvs = [np.ascontiguousarray(V[b].T).astype(h).reshape(IC, 128, S)
           for b in range(B)]
    WqT, WkT, WvT, WoT = Wq.T, Wk.T, Wv.T, Wo.T
    bob = np.ascontiguousarray(np.broadcast_to(bo / 4.0, (128, D)), dtype=f)

    in_maps = []
    for c in range(NCORES):
        b, g = c // 4, c % 4
        cols = slice(DG * g, DG * (g + 1))
        in_maps.append({
            "xq": xqs[b], "xk": xks[b], "xv": xvs[b],
            "wq": np.ascontiguousarray(WqT[:, cols], dtype=h).reshape(IC, 128, DG),
            "wk": np.ascontiguousarray(WkT[:, cols], dtype=h).reshape(IC, 128, DG),
            "wv": np.ascontiguousarray(WvT[:, cols], dtype=h).reshape(IC, 128, DG),
            "wo": np.ascontiguousarray(WoT[cols, :], dtype=h).reshape(2, 128, D),
            "bq2": np.ascontiguousarray(bq[cols]).reshape(2, 128),
            "bk2": np.ascontiguousarray(bk[cols]).reshape(2, 128),
            "bvb": np.ascontiguousarray(np.broadcast_to(bv[cols], (128, DG))),
            "bob": bob,
            "ones1": _ONES,
        })
    return in_maps


def _assemble(results):
    out = np.empty((B, S, D), np.float32)
    for c in range(NCORES):
        b, g = c // 4, c % 4
        for j in range(4):
            out[b, 512 * j + 128 * g:512 * j + 128 * (g + 1), :] = \
                results[c]["out"][j]
    return out


def kernel(**inputs):
    nc = _get_nc()
    in_maps = _prep_inputs(**inputs)
    res = run_bass_kernel_spmd(nc, in_maps, core_ids=list(range(NCORES)))
    return _assemble(res.results)


# revision 10
# speedup vs baseline: 1.4504x; 1.0183x over previous
"""MultiHeadAttention (B=2, S=2048, D=1024, H=16) on 8 Trainium2 NeuronCores.

Sharding: core c -> batch b = c // 4, head group g = c % 4 (4 of 16 heads =
256 of the 1024 projection columns). Within a batch's 4-core group:

  P1-P2: q/k projections for the core's 4 heads over the full sequence,
         produced directly transposed/head-major: qT,kT [256, S] fp16.
  P3:    v projection in natural layout [S, 4*65] fp16, with a ones column
         appended per head (yields softmax denominators for free in P4).
  P4:    per head and 1024-wide query slice: scoresT[k,q] = kT_blk.T @ qT
         (fp16 operands, fp32 PSUM), one 1024-wide exp on ScalarE with the
         1/sqrt(64) scale folded in (scores are O(10): no max pass needed),
         then oT'[65,q] += v_blk.T @ pT. Row 64 of oT' is the softmax sum;
         normalize via reciprocal + PE broadcast -> ctxT [256, S] fp16.
  P5:    partial output projection for the finished 1024 query rows
         (+ bo/4 so the group sum applies bo exactly once).
  P6:    ReduceScatter(add) in 512-row chunks, pipelined behind P4/P5 so
         only the last chunk's exchange is serial; rank r of the group
         receives rows [512j + 128r, +128) of chunk j.

Matmul operands are fp16 (1 cycle/row on the PE; same 10-bit mantissa class
as the fp32r mode, which measures ~2 cycles/row); all accumulation is fp32.
"""

import numpy as np

import concourse.bacc as bacc
import concourse.mybir as mybir
from concourse.tile import TileContext
from concourse.bass_utils import run_bass_kernel_spmd

F32 = mybir.dt.float32
F32R = mybir.dt.float32r
F16 = mybir.dt.float16

B, S, D = 2, 2048, 1024
H, DH = 16, 64
NCORES = 8
GROUPS = [[0, 1, 2, 3], [4, 5, 6, 7]]
HPG = 4            # heads per core
DG = HPG * DH      # 256 projection cols per core
IC = D // 128      # 8 contraction chunks for the projections
KC = S // 128      # 16 key blocks
VW = DH + 1        # 65 = head dim + ones column

_NC_CACHE = {}
_ONES = np.ones((1, DH), np.float32)


def _build_nc():
    nc = bacc.Bacc("TRN2", target_bir_lowering=False, num_devices=NCORES)

    xq = nc.dram_tensor("xq", [IC, 128, S], F16, kind="ExternalInput")
    xk = nc.dram_tensor("xk", [IC, 128, S], F16, kind="ExternalInput")
    xv = nc.dram_tensor("xv", [IC, 128, S], F16, kind="ExternalInput")
    wq = nc.dram_tensor("wq", [IC, 128, DG], F16, kind="ExternalInput")
    wk = nc.dram_tensor("wk", [IC, 128, DG], F16, kind="ExternalInput")
    wv = nc.dram_tensor("wv", [IC, 128, DG], F16, kind="ExternalInput")
    wo = nc.dram_tensor("wo", [2, 128, D], F16, kind="ExternalInput")
    bq2 = nc.dram_tensor("bq2", [2, 128], F32, kind="ExternalInput")
    bk2 = nc.dram_tensor("bk2", [2, 128], F32, kind="ExternalInput")
    bvb = nc.dram_tensor("bvb", [128, DG], F32, kind="ExternalInput")
    bob = nc.dram_tensor("bob", [128, D], F32, kind="ExternalInput")
    ones1 = nc.dram_tensor("ones1", [1, DH], F32R, kind="ExternalInput")
    out = nc.dram_tensor("out", [4, 128, D], F32, kind="ExternalOutput")

    partial = nc.dram_tensor("partial", [S, D], F32)
    rs_out = nc.dram_tensor("rs_out", [4, 128, D], F32)

    Exp = mybir.ActivationFunctionType.Exp

    with nc.allow_low_precision(reason="fp16 attention internals"), \
            TileContext(nc) as tc:
        with (
            tc.tile_pool(name="persist", bufs=1) as pers,
            tc.tile_pool(name="xin", bufs=3) as xin,
            tc.tile_pool(name="pt", bufs=4) as ptp,
            tc.tile_pool(name="small", bufs=4) as small,
            tc.tile_pool(name="outp", bufs=3) as outp,
            tc.tile_pool(name="ps", bufs=2, space="PSUM") as ps,
            tc.tile_pool(name="ps2", bufs=3, space="PSUM") as ps2,
            tc.tile_pool(name="psov", bufs=3, space="PSUM") as psov,
        ):
            # ---- persistent SBUF ----
            wq_sb = pers.tile([128, IC * DG], F16, tag="wq")
            wk_sb = pers.tile([128, IC * DG], F16, tag="wk")
            wv_sb = pers.tile([128, IC * DG], F16, tag="wv")
            wo_sb = pers.tile([128, 2 * D], F16, tag="wo")
            qt_sb = [pers.tile([128, S], F16, tag=f"qt{i}", name=f"qt{i}")
                     for i in range(2)]
            kt_sb = [pers.tile([128, S], F16, tag=f"kt{i}", name=f"kt{i}")
                     for i in range(2)]
            v_sb = pers.tile([128, KC * HPG * VW], F16, tag="v")
            ctxT_sb = [pers.tile([128, S], F16, tag=f"cx{i}", name=f"cx{i}")
                       for i in range(2)]
            bq_sb = pers.tile([128, 2], F32, tag="bq")
            bk_sb = pers.tile([128, 2], F32, tag="bk")
            bvb_sb = pers.tile([128, DG], F32, tag="bvb")
            bob_sb = pers.tile([128, D], F32, tag="bob")
            ones_sb = pers.tile([1, DH], F32R, tag="ones")

            for wsb, wdr, kchunks in (
                (wq_sb, wq, IC), (wk_sb, wk, IC), (wv_sb, wv, IC), (wo_sb, wo, 2),
            ):
                nc.sync.dma_start(
                    out=wsb.rearrange("p (k n) -> p k n", k=kchunks),
                    in_=wdr.rearrange("k p n -> p k n"),
                )
            nc.sync.dma_start(out=bq_sb[:], in_=bq2.rearrange("c p -> p c"))
            nc.sync.dma_start(out=bk_sb[:], in_=bk2.rearrange("c p -> p c"))
            nc.sync.dma_start(out=bvb_sb[:], in_=bvb[:])
            nc.sync.dma_start(out=bob_sb[:], in_=bob[:])
            nc.sync.dma_start(out=ones_sb[:], in_=ones1[:])
            nc.vector.memset(
                v_sb.rearrange("p (k h e) -> p k h e", h=HPG, e=VW)[:, :, :, DH], 1.0
            )

            # ---- P1/P2: k then q projections -> head-major [256, S] fp16 ----
            for xdr, wsb, bsb, dst in (
                (xk, wk_sb, bk_sb, kt_sb),
                (xq, wq_sb, bq_sb, qt_sb),
            ):
                wsb3 = wsb.rearrange("p (k n) -> p k n", k=IC)
                for s4 in range(4):  # 512-wide sequence slice
                    xt = xin.tile([128, IC * 512], F16, tag="x")
                    nc.sync.dma_start(
                        out=xt.rearrange("p (k n) -> p k n", k=IC),
                        in_=xdr[:, :, s4 * 512:(s4 + 1) * 512].rearrange(
                            "k p n -> p k n"),
                    )
                    xt3 = xt.rearrange("p (k n) -> p k n", k=IC)
                    for oc in range(2):
                        acc = ps.tile([128, 512], F32, tag="mm")
                        for ic in range(IC):
                            nc.tensor.matmul(
                                acc[:],
                                wsb3[:, ic, oc * 128:(oc + 1) * 128],
                                xt3[:, ic, :],
                                start=(ic == 0),
                                stop=(ic == IC - 1),
                            )
                        nc.vector.tensor_scalar_add(
                            dst[oc][:, s4 * 512:(s4 + 1) * 512],
                            acc[:],
                            bsb[:, oc:oc + 1],
                        )

            # ---- P3: v projection -> [S, 4*65] fp16 with ones columns ----
            wv3 = wv_sb.rearrange("p (k n) -> p k n", k=IC)
            v4 = v_sb.rearrange("p (k h e) -> p k h e", h=HPG, e=VW)
            for s4 in range(4):
                xt = xin.tile([128, IC * 512], F16, tag="x")
                nc.sync.dma_start(
                    out=xt.rearrange("p (k n) -> p k n", k=IC),
                    in_=xv[:, :, s4 * 512:(s4 + 1) * 512].rearrange("k p n -> p k n"),
                )
                xt3 = xt.rearrange("p (k n) -> p k n", k=IC)
                for j in range(4):  # key chunk kc = 4*s4 + j
                    kc = 4 * s4 + j
                    acc = ps.tile([128, 512], F32, tag="mm")
                    for ic in range(IC):
                        nc.tensor.matmul(
                            acc[:, 0:DG],
                            xt3[:, ic, j * 128:(j + 1) * 128],
                            wv3[:, ic, :],
                            start=(ic == 0),
                            stop=(ic == IC - 1),
                        )
                    nc.vector.tensor_add(
                        out=v4[:, kc, :, 0:DH],
                        in0=acc[:, 0:DG].rearrange("p (h e) -> p h e", e=DH),
                        in1=bvb_sb.rearrange("p (h e) -> p h e", e=DH),
                    )

            # ---- P4 + P5 + P6, pipelined per 512-wide query slice ----
            wo3 = wo_sb.rearrange("p (k n) -> p k n", k=2)
            for qs in range(4):  # 512-wide query slice == 512-row RS chunk
                qlo = qs * 512
                for hg in range(HPG):
                    oc, ofs = hg // 2, 64 * (hg % 2)
                    ov = psov.tile([VW, 512], F32, tag="ov")
                    for kb in range(KC):
                        sc = ps2.tile([128, 512], F32, tag="sc")
                        nc.tensor.matmul(
                            sc[:],
                            kt_sb[oc][ofs:ofs + DH, kb * 128:(kb + 1) * 128],
                            qt_sb[oc][ofs:ofs + DH, qlo:qlo + 512],
                            start=True,
                            stop=True,
                        )
                        pt = ptp.tile([128, 512], F16, tag="pt")
                        nc.scalar.activation(pt[:], sc[:], Exp, scale=0.125)
                        nc.tensor.matmul(
                            ov[:],
                            v_sb[:, (kb * HPG + hg) * VW:
                                 (kb * HPG + hg + 1) * VW],
                            pt[:],
                            start=(kb == 0),
                            stop=(kb == KC - 1),
                        )
                    recip = small.tile([1, 512], F32R, tag="rc")
                    nc.vector.reciprocal(recip[:], ov[DH:VW, :])
                    bc = ps.tile([DH, 512], F32, tag="mm")
                    nc.tensor.matmul(bc[:], ones_sb[:], recip[:],
                                     start=True, stop=True)
                    bcs = small.tile([DH, 512], F32, tag="bcs")
                    nc.vector.tensor_copy(out=bcs[:], in_=bc[:])
                    nc.vector.tensor_mul(
                        out=ctxT_sb[oc][ofs:ofs + DH, qlo:qlo + 512],
                        in0=ov[0:DH, :],
                        in1=bcs[:],
                    )

                # P5: output projection for the finished 512 rows, then
                # ReduceScatter for this chunk (hidden under the next slice's
                # attention except for the last chunk).
                for ibl in range(4):
                    ib = 4 * qs + ibl
                    for oh in range(2):
                        acc = ps.tile([128, 512], F32, tag="mm")
                        for cc in range(2):
                            nc.tensor.matmul(
                                acc[:],
                                ctxT_sb[cc][:, ib * 128:(ib + 1) * 128],
                                wo3[:, cc, oh * 512:(oh + 1) * 512],
                                start=(cc == 0),
                                stop=(cc == 1),
                            )
                        ot = outp.tile([128, 512], F32, tag="ot")
                        nc.vector.tensor_add(
                            out=ot[:], in0=acc[:],
                            in1=bob_sb[:, oh * 512:(oh + 1) * 512],
                        )
                        nc.sync.dma_start(
                            out=partial[ib * 128:(ib + 1) * 128,
                                        oh * 512:(oh + 1) * 512],
                            in_=ot[:],
                        )
                nc.gpsimd.collective_compute(
                    "ReduceScatter",
                    mybir.AluOpType.add,
                    replica_groups=GROUPS,
                    ins=[partial[qs * 512:(qs + 1) * 512, :]],
                    outs=[rs_out[qs]],
                )
                nc.sync.dma_start(out=out[qs], in_=rs_out[qs])

    nc.compile()
    return nc


def _get_nc():
    if "nc" not in _NC_CACHE:
        _NC_CACHE["nc"] = _build_nc()
    return _NC_CACHE["nc"]


def _prep_inputs(Q, K, V, Wq, Wk, Wv, Wo, bq, bk, bv, bo):
    f = np.float32
    h = np.float16
    Q, K, V = (np.asarray(a, f) for a in (Q, K, V))
    Wq, Wk, Wv, Wo = (np.asarray(a, f) for a in (Wq, Wk, Wv, Wo))
    bq, bk, bv, bo = (np.asarray(a, f) for a in (bq, bk, bv, bo))

    xqs = [np.ascontiguousarray(Q[b].T).astype(h).reshape(IC, 128, S)
           for b in range(B)]
    xks = [np.ascontiguousarray(K[b].T).astype(h).reshape(IC, 128, S)
           for b in range(B)]
    xvs = [np.ascontiguousarray(V[b].T).astype(h).reshape(IC, 128, S)
           for b in range(B)]
    WqT, WkT, WvT, WoT = Wq.T, Wk.T, Wv.T, Wo.T
    bob = np.ascontiguousarray(np.broadcast_to(bo / 4.0, (128, D)), dtype=f)

    in_maps = []
    for c in range(NCORES):
        b, g = c // 4, c % 4
        cols = slice(DG * g, DG * (g + 1))
        in_maps.append({
            "xq": xqs[b], "xk": xks[b], "xv": xvs[b],
            "wq": np.ascontiguousarray(WqT[:, cols], dtype=h).reshape(IC, 128, DG),
            "wk": np.ascontiguousarray(WkT[:, cols], dtype=h).reshape(IC, 128, DG),
            "wv": np.ascontiguousarray(WvT[:, cols], dtype=h).reshape(IC, 128, DG),
            "wo": np.ascontiguousarray(WoT[cols, :], dtype=h).reshape(2, 128, D),
            "bq2": np.ascontiguousarray(bq[cols]).reshape(2, 128),
            "bk2": np.ascontiguousarray(bk[cols]).reshape(2, 128),
            "bvb": np.ascontiguousarray(np.broadcast_to(bv[cols], (128, DG))),
            "bob": bob,
            "ones1": _ONES,
        })
    return in_maps


def _assemble(results):
    out = np.empty((B, S, D), np.float32)
    for c in range(NCORES):
        b, g = c // 4, c % 4
        for j in range(4):
            out[b, 512 * j + 128 * g:512 * j + 128 * (g + 1), :] = \
                results[c]["out"][j]
    return out


def kernel(**inputs):
    nc = _get_nc()
    in_maps = _prep_inputs(**inputs)
    res = run_bass_kernel_spmd(nc, in_maps, core_ids=list(range(NCORES)))
    return _assemble(res.results)
